# revision 16
# baseline (speedup 1.0000x reference)
"""CISS-VAE (per-cluster MoE-routed MLP chain) Trainium2 kernel.

Strategy (routing on host, compute on device):
  - Rows are grouped by cluster label on the host. Core c processes all rows
    of cluster c (C == n_cores == 8), so every GEMM on the device is a dense
    per-cluster GEMM.
  - The encoder (enc0, encu, enc2, mu, lv) runs in fp8-e4m3 with DoubleRow
    perf mode (2x PE throughput). This is numerically safe because the VAE
    latent z = mu + exp(0.5*logvar)*eps is dominated by the exact eps input:
    quantizing the whole encoder to fp8 moves the final output by ~2e-4
    relative (measured against the f32 reference on the real input stats).
  - The decoder (dec0, dec1, dec2, fin) stays bf16: fp8 there costs 1-2e-2
    relative error, too close to the tolerance.
  - fp8 operands are pre-scaled by powers of two (exact): x by 2^4, encoder
    weights by 2^8, hidden activations by 2^4. The descale (2^-8 per layer)
    is folded into the PSUM eviction's activation scale.
  - All tensors on device are feature-major; k-tiles are packed in the free
    dimension ([128, n_ktiles, rows]) so a DoubleRow matmul can consume two
    k-tiles per instruction via a 3-d access pattern.
  - Per-feature biases live on partitions and are fused into the PSUM->SBUF
    eviction (Relu/Identity/Exp). The encoder needs scale+bias+relu (3 alu
    stages) so its evictions go to the Scalar engine; decoder evictions
    alternate Scalar/Vector.
  - The encoder is eviction-bound (a PSUM group every ~220ns vs ~650ns per
    Scalar eviction), so encoder m-tile groups of block b are WOVEN with
    decoder groups of block b-1: the decoder's long bf16 matmul groups give
    the Scalar engine time to drain encoder PSUMs. The last ~6 decoder
    groups run after all encoder groups so the mu/lv evictions and the
    z-chain (DVE) finish under decoder matmul time, and x/eps loads are
    prefetched a full block ahead.
  - Row blocks are 512 rows; the last block absorbs the remainder (up to
    1023 rows, PSUM-segmented at 512) to avoid a low-efficiency tiny block.
  - x / out DRAM layouts are block-contiguous so the per-block DMAs are
    single contiguous transfers (inputs on the sync HWDGE queue, output
    stores on the otherwise-idle gpsimd SWDGE queue).
"""

import ml_dtypes
import numpy as np

import concourse.bacc as bacc
import concourse.mybir as mybir
import concourse.tile as tile
from concourse import bass_utils

P = 128
D_IN, LAT, C = 512, 64, 8
H0, H1, H2 = 1024, 512, 256
N_CORES = 8
KX = D_IN // P
F32 = mybir.dt.float32
BF16 = mybir.dt.bfloat16
FP8 = mybir.dt.float8e4
AF = mybir.ActivationFunctionType
ALU = mybir.AluOpType
DR = mybir.MatmulPerfMode.DoubleRow
BF16_NP = ml_dtypes.bfloat16
FP8_NP = ml_dtypes.float8_e4m3

SX = 1.0 / 16.0   # fp8 scale on x (chosen so enc0's eviction scale is 1.0)
SW = 256.0        # fp8 scale on encoder weights
SA = 16.0         # fp8 scale on hidden activations
SEV0 = SA / (SX * SW)  # enc0 eviction scale == 1.0 -> 2-op DVE eligible
SEV = SA / (SA * SW)   # encu/enc2 eviction scale (2^-8)

# layer table: name -> (f_in, f_out, fp8)
LAYERS = dict(
    enc0=(D_IN, H0, True),
    encu=(H0, H1, True),
    enc2=(H1, H2, True),
    mu=(H2, LAT, True),
    lv=(H2, LAT, True),
    dec0=(LAT, H2, False),
    dec1=(H2, H1, False),
    dec2=(H1, H0, False),
    fin=(H0, D_IN, False),
)

DEC_TAIL = 4  # decoder groups emitted after the woven section


def _ceil_to(x, m):
    return ((x + m - 1) // m) * m


def _segs(nb, m):
    out = [m] * (nb // m)
    if nb % m:
        out.append(nb % m)
    return out


def _b2d(b, scale=1.0):
    """[f] bias -> [min(f,128), n_mtiles] (partition-major per m-tile)."""
    b = np.asarray(b, dtype=np.float32) * scale
    f = b.shape[0]
    if f >= P:
        return np.ascontiguousarray(b.reshape(f // P, P).T.astype(np.float32))
    return np.ascontiguousarray(b.reshape(1, f).T.astype(np.float32))


def _wpack(W, dt_np, scale=1.0):
    """[fi, fo] -> [min(fi,128), kt*fo] k-tile packed, cast to dt_np."""
    W = np.asarray(W, dtype=np.float32) * scale
    fi, fo = W.shape
    kp = min(P, fi)
    kt = max(1, fi // P)
    Wp = W.reshape(kt, kp, fo).transpose(1, 0, 2)
    Wp = np.clip(Wp, -240, 240).astype(dt_np)
    return np.ascontiguousarray(Wp.reshape(kp, kt * fo))


def _weave(*lists):
    """Merge lists of closures, interleaving by fractional progress."""
    lists = [l for l in lists if l]
    idx = [0] * len(lists)
    out = []
    while True:
        best, bp = -1, 2.0
        for j, l in enumerate(lists):
            if idx[j] < len(l):
                p = idx[j] / len(l)
                if p < bp:
                    best, bp = j, p
        if best < 0:
            return out
        out.append(lists[best][idx[best]])
        idx[best] += 1


def _build_module(npad, blocks):
    nc = bacc.Bacc("TRN2", target_bir_lowering=False, debug=False)

    dram = {}

    def din(name, shape, dt):
        dram[name] = nc.dram_tensor(name, list(shape), dt, kind="ExternalInput").ap()
        return dram[name]

    xT = din("xT", (P, KX * npad), FP8)
    epsT = din("epsT", (LAT, npad), F32)

    for name, (fi, fo, fp8) in LAYERS.items():
        kp = min(P, fi)
        kt = max(1, fi // P)
        din("w_" + name, (kp, kt * fo), FP8 if fp8 else BF16)
        din("b_" + name, (P if fo >= P else fo, max(1, fo // P)), F32)

    outT = nc.dram_tensor("outT", [P, KX * npad], F32, kind="ExternalOutput").ap()

    with tile.TileContext(nc) as tc:
        with (
            tc.tile_pool(name="wpool", bufs=1) as wpool,
            tc.tile_pool(name="acts", bufs=2) as acts,
            tc.tile_pool(name="psum", bufs=8, space="PSUM") as psum,
        ):
            wsb = {}
            bsb = {}
            dma_rr = [0]

            def prologue_dma(out, in_):
                eng = nc.sync if dma_rr[0] % 2 == 0 else nc.scalar
                dma_rr[0] += 1
                eng.dma_start(out, in_)

            DEC_W = ("dec0", "dec1", "dec2", "fin")

            def load_weights(name):
                if name in wsb:
                    return
                fi, fo, fp8 = LAYERS[name]
                kp = min(P, fi)
                kt = max(1, fi // P)
                dt = FP8 if fp8 else BF16
                w_t = wpool.tile([kp, kt, fo], dt, tag=f"w_{name}", name=f"w_{name}")
                src = dram["w_" + name].rearrange("p (k f) -> p k f", k=kt)
                # spread encoder weight loads across the two input queues so
                # block 0's layers never wait behind a single queue
                weng = dict(enc0=nc.sync, encu=nc.scalar, enc2=nc.sync,
                            mu=nc.scalar, lv=nc.scalar)
                if name in DEC_W:
                    nc.gpsimd.dma_start(w_t[:], src)
                else:
                    weng[name].dma_start(w_t[:], src)
                bp = P if fo >= P else fo
                b_t = wpool.tile([bp, max(1, fo // P)], F32, tag=f"b_{name}", name=f"b_{name}")
                nc.gpsimd.dma_start(b_t[:], dram["b_" + name][:])
                wsb[name] = w_t
                bsb[name] = b_t

            def groups_fp8(lname, in_t, nb, func, scale, out_dt, evict="act"):
                """DoubleRow fp8 GEMM out = func(scale*(W.T @ in) + b).
                Returns (out_tile, [emit closures, one per m-tile])."""
                load_weights(lname)
                fi, fo, _ = LAYERS[lname]
                w_t, b_t = wsb[lname], bsb[lname]
                kt = fi // P
                n_m = max(1, fo // P)
                mp = min(P, fo)
                if fo >= P:
                    o_t = acts.tile([P, n_m, nb], out_dt, tag=f"h_{lname}", name=f"h_{lname}")
                else:
                    o_t = acts.tile([fo, nb], out_dt, tag=f"h_{lname}", name=f"h_{lname}")

                def mk(m):
                    def emit():
                        bias = b_t[:mp, m : m + 1]
                        s0 = 0
                        for seg in _segs(nb, 512):
                            ps = psum.tile([mp, seg], F32, tag="ps", name=f"ps_{lname}_{m}")
                            nkp = kt // 2
                            c0 = 0
                            for j, csz in enumerate(_segs(seg, 256)):
                                for i in range(nkp):
                                    nc.tensor.matmul(
                                        ps[:, c0 : c0 + csz],
                                        w_t[:, 2 * i : 2 * i + 2, m * mp : (m + 1) * mp],
                                        in_t[:, 2 * i : 2 * i + 2, s0 + c0 : s0 + c0 + csz],
                                        start=(i == 0),
                                        stop=(i == nkp - 1),
                                        perf_mode=DR,
                                    )
                                c0 += csz
                            dst = (o_t[:, m, s0 : s0 + seg] if fo >= P
                                   else o_t[:, s0 : s0 + seg])
                            if evict == "alt1" and m % 2 == 1:
                                # scale == 1.0: relu on DVE in 2 alu stages
                                nc.vector.tensor_scalar(dst, ps[:], bias, 0.0, ALU.add, ALU.max)
                            elif evict in ("act", "alt1"):
                                nc.scalar.activation(dst, ps[:], func, bias=bias, scale=scale)
                            else:  # dve identity: (ps * scale) + bias
                                nc.vector.tensor_scalar(dst, ps[:], scale, bias, ALU.mult, ALU.add)
                            s0 += seg
                    return emit

                return o_t, [mk(m) for m in range(n_m)]

            def groups_bf16(lname, in_t, nb, func, out_dt=BF16, out_tag=None):
                """bf16 GEMM; returns (out_tile, [closures])."""
                load_weights(lname)
                fi, fo, _ = LAYERS[lname]
                w_t, b_t = wsb[lname], bsb[lname]
                kt = max(1, fi // P)
                n_m = max(1, fo // P)
                mp = min(P, fo)
                tag = out_tag or f"h_{lname}"
                o_t = acts.tile([P, n_m, nb], out_dt, tag=tag, name=tag)

                def mk(m):
                    def emit():
                        bias = b_t[:mp, m : m + 1]
                        s0 = 0
                        for seg in _segs(nb, 512):
                            ps = psum.tile([mp, seg], F32, tag="ps", name=f"ps_{lname}_{m}")
                            for k in range(kt):
                                mov = (in_t[:, k, s0 : s0 + seg] if kt > 1
                                       else in_t[:, s0 : s0 + seg])
                                nc.tensor.matmul(
                                    ps[:],
                                    w_t[:, k, m * mp : (m + 1) * mp],
                                    mov,
                                    start=(k == 0),
                                    stop=(k == kt - 1),
                                )
                            dst = o_t[:, m, s0 : s0 + seg]
                            if m % 2 == 1:
                                if func is AF.Relu:
                                    nc.vector.tensor_scalar(dst, ps[:], bias, 0.0, ALU.add, ALU.max)
                                else:
                                    nc.vector.tensor_scalar(dst, ps[:], bias, None, ALU.add)
                            else:
                                nc.scalar.activation(
                                    dst, ps[:], func if func is not None else AF.Identity,
                                    bias=bias, scale=1.0,
                                )
                            s0 += seg
                    return emit

                return o_t, [mk(m) for m in range(n_m)]

            n_blk = len(blocks)
            offs = [sum(blocks[:i]) for i in range(n_blk)]
            x_in = [None] * n_blk
            eps_in = [None] * n_blk
            mu_sg = [None] * n_blk
            lat_out = [None] * n_blk

            def stage_load(b):
                nb, off = blocks[b], offs[b]
                x_t = acts.tile([P, KX, nb], FP8, tag="x", bufs=3, name="x")
                src = xT[:, KX * off : KX * (off + nb)].rearrange("p (k n) -> p k n", k=KX)
                (prologue_dma if b == 0 else nc.sync.dma_start)(x_t[:], src)
                e_t = acts.tile([LAT, nb], F32, tag="eps", bufs=3, name="e_t")
                (prologue_dma if b == 0 else nc.sync.dma_start)(e_t[:], epsT[:, off : off + nb])
                x_in[b], eps_in[b] = x_t, e_t

            def build_enc(b):
                nb = blocks[b]
                h0, g0 = groups_fp8("enc0", x_in[b], nb, AF.Relu, SEV0, FP8, evict="alt1")
                h1, g1 = groups_fp8("encu", h0, nb, AF.Relu, SEV, FP8)
                h2, g2 = groups_fp8("enc2", h1, nb, AF.Relu, SEV, FP8)
                mu, gm = groups_fp8("mu", h2, nb, None, 1.0 / (SA * SW), F32, evict="dve")
                sg, gl = groups_fp8("lv", h2, nb, AF.Exp, 0.5 / (SA * SW), F32)
                mu_sg[b] = (mu, sg)
                return g0 + g1 + g2 + gm + gl

            def stage_lat(b):
                nb = blocks[b]
                mu, sg = mu_sg[b]
                tmp = acts.tile([LAT, nb], F32, tag="tmp", bufs=2, name="tmp")
                nc.vector.tensor_mul(tmp[:], sg[:], eps_in[b][:])
                z = acts.tile([LAT, nb], BF16, tag="z", bufs=2, name="z")
                nc.vector.tensor_add(z[:], tmp[:], mu[:])
                lat_out[b] = z

            def build_dec(b):
                nb, off = blocks[b], offs[b]
                h3, g3 = groups_bf16("dec0", lat_out[b], nb, AF.Relu)
                h4, g4 = groups_bf16("dec1", h3, nb, AF.Relu)
                h5, g5 = groups_bf16("dec2", h4, nb, AF.Relu)
                ot, g6 = groups_bf16("fin", h5, nb, None, out_dt=F32, out_tag="out")

                # store each fin m-tile right after its eviction so the output
                # DMA pipelines with the remaining fin groups
                def mk_store(m):
                    def store():
                        eng = nc.sync if m % 2 == 0 else nc.gpsimd
                        eng.dma_start(
                            outT[:, KX * off + m * nb : KX * off + (m + 1) * nb],
                            ot[:, m, :],
                        )
                    return store

                g6s = []
                for m, g in enumerate(g6):
                    g6s += [g, mk_store(m)]
                return g3 + g4 + g5 + g6s

            # Warm up the PE (clock gate) with dummy matmuls while the
            # prologue DMAs stream in; rotate psum banks so they pipeline.
            wu_w = wpool.tile([P, P], BF16, tag="wu_w", name="wu_w")
            wu_x = wpool.tile([P, 512], BF16, tag="wu_x", name="wu_x")
            nc.vector.memset(wu_w[:], 0.0)
            nc.vector.memset(wu_x[:], 0.0)
            wu_ps = [psum.tile([P, 512], F32, tag="ps", name=f"wu_ps{i}") for i in range(3)]
            for i in range(15):
                nc.tensor.matmul(wu_ps[i % 3][:], wu_w[:], wu_x[:], start=True, stop=True)

            # software pipeline: weave encoder groups of block b with decoder
            # groups of block b-1; the decoder tail runs after the encoder so
            # mu/lv evictions + the z chain hide under decoder matmuls.
            # Load order on the input queues: x/eps(0) first, then encoder
            # weights, then the block-1 prefetch -- so block 0 never waits.
            stage_load(0)
            for name in ("enc0", "encu", "enc2", "mu", "lv"):
                load_weights(name)
            if n_blk > 1:
                stage_load(1)
            for g in build_enc(0):
                g()
            stage_lat(0)
            for b in range(1, n_blk):
                if b + 1 < n_blk:
                    stage_load(b + 1)
                dec = build_dec(b - 1)
                enc = build_enc(b)
                # block 1 only: give the encoder a head start, since block 0
                # has no decoder tail to hide its z-chain latency
                head = 4 if b == 1 else 0
                for g in enc[:head]:
                    g()
                for g in _weave(enc[head:], dec[:-DEC_TAIL]):
                    g()
                stage_lat(b)
                for g in dec[-DEC_TAIL:]:
                    g()
            for g in build_dec(n_blk - 1):
                g()

    nc.compile()
    return nc


def kernel(**inputs):
    x = np.asarray(inputs["x"], dtype=np.float32)
    lbl = np.asarray(inputs["cluster_labels"]).astype(np.int64)
    eps = np.asarray(inputs["eps"], dtype=np.float32)
    B = x.shape[0]

    counts = np.bincount(lbl, minlength=C)
    npad = max(512, _ceil_to(int(counts.max()), 64))
    n_full = max(0, npad // 512 - 1)
    blocks = [512] * n_full + [npad - 512 * n_full]

    rows = [np.nonzero(lbl == c)[0] for c in range(C)]
    offs = [sum(blocks[:i]) for i in range(len(blocks))]

    shared = {
        "w_enc0": _wpack(inputs["enc_W0"], FP8_NP, SW),
        "b_enc0": _b2d(inputs["enc_b0"], SA),
        "w_enc2": _wpack(inputs["enc_W2"], FP8_NP, SW),
        "b_enc2": _b2d(inputs["enc_b2"], SA),
        "w_mu": _wpack(inputs["mu_W"], FP8_NP, SW),
        "b_mu": _b2d(inputs["mu_b"]),
        "w_lv": _wpack(inputs["lv_W"], FP8_NP, SW),
        "b_lv": _b2d(inputs["lv_b"], 0.5),
        "w_dec1": _wpack(inputs["dec_W1"], BF16_NP),
        "b_dec1": _b2d(inputs["dec_b1"]),
    }

    in_maps = []
    for c in range(C):
        r = rows[c]
        xq = np.zeros((P, KX * npad), FP8_NP)
        xs = np.clip(
            x[r].T.reshape(KX, P, len(r)).transpose(1, 0, 2) * SX, -240, 240
        ).astype(FP8_NP)
        # block-contiguous layout: block b occupies cols [KX*off, KX*(off+nb))
        # with k-tile stride nb inside the block
        for off, nb in zip(offs, blocks):
            lo, hi = off, min(off + nb, len(r))
            if lo >= len(r):
                break
            seg = np.zeros((P, KX, nb), FP8_NP)
            seg[:, :, : hi - lo] = xs[:, :, lo:hi]
            xq[:, KX * off : KX * (off + nb)] = seg.reshape(P, -1)
        epsT = np.zeros((LAT, npad), np.float32)
        epsT[:, : len(r)] = eps[r].T
        m = dict(shared)
        m["xT"] = xq
        m["epsT"] = epsT
        for nm, W, b, bs in (
            ("encu", inputs["enc_Wu"][c], inputs["enc_bu"][c], SA),
            ("dec0", inputs["dec_Wu0"][c], inputs["dec_bu0"][c], 1.0),
            ("dec2", inputs["dec_Wu2"][c], inputs["dec_bu2"][c], 1.0),
            ("fin", inputs["fin_W"][c], inputs["fin_b"][c], 1.0),
        ):
            fp8 = LAYERS[nm][2]
            m["w_" + nm] = _wpack(W, FP8_NP if fp8 else BF16_NP, SW if fp8 else 1.0)
            m["b_" + nm] = _b2d(b, bs)
        in_maps.append(m)

    nc = _build_module(npad, blocks)
    res = bass_utils.run_bass_kernel_spmd(nc, in_maps, core_ids=list(range(N_CORES)))
    global LAST_RESULTS
    LAST_RESULTS = res

    out = np.empty((B, D_IN), np.float32)
    for c in range(C):
        r = rows[c]
        arr = res.results[c]["outT"]
        for off, nb in zip(offs, blocks):
            lo, hi = off, min(off + nb, len(r))
            if lo >= len(r):
                break
            seg = arr[:, KX * off : KX * (off + nb)].reshape(P, KX, nb)[:, :, : hi - lo]
            out[r[lo:hi]] = seg.transpose(2, 1, 0).reshape(hi - lo, D_IN)
    return out


# revision 21
# speedup vs baseline: 1.0787x; 1.0787x over previous
"""CISS-VAE (per-cluster MoE-routed MLP chain) Trainium2 kernel.

Strategy (routing on host, compute on device):
  - Rows are grouped by cluster label on the host. Core c processes all rows
    of cluster c (C == n_cores == 8), so every GEMM on the device is a dense
    per-cluster GEMM.
  - The encoder (enc0, encu, enc2, mu, lv) runs in fp8-e4m3 with DoubleRow
    perf mode (2x PE throughput). This is numerically safe because the VAE
    latent z = mu + exp(0.5*logvar)*eps is dominated by the exact eps input:
    quantizing the whole encoder to fp8 moves the final output by ~2e-4
    relative (measured against the f32 reference on the real input stats).
  - The decoder (dec0, dec1, dec2, fin) stays bf16: fp8 there costs 1-2e-2
    relative error, too close to the tolerance.
  - fp8 operands are pre-scaled by powers of two (exact): x by 2^4, encoder
    weights by 2^8, hidden activations by 2^4. The descale (2^-8 per layer)
    is folded into the PSUM eviction's activation scale.
  - All tensors on device are feature-major; k-tiles are packed in the free
    dimension ([128, n_ktiles, rows]) so a DoubleRow matmul can consume two
    k-tiles per instruction via a 3-d access pattern.
  - Per-feature biases live on partitions and are fused into the PSUM->SBUF
    eviction (Relu/Identity/Exp). The encoder needs scale+bias+relu (3 alu
    stages) so its evictions go to the Scalar engine; decoder evictions
    alternate Scalar/Vector.
  - The encoder is eviction-bound (a PSUM group every ~220ns vs ~650ns per
    Scalar eviction), so encoder m-tile groups of block b are WOVEN with
    decoder groups of block b-1: the decoder's long bf16 matmul groups give
    the Scalar engine time to drain encoder PSUMs. The last ~6 decoder
    groups run after all encoder groups so the mu/lv evictions and the
    z-chain (DVE) finish under decoder matmul time, and x/eps loads are
    prefetched a full block ahead.
  - Row blocks are 512 rows; the last block absorbs the remainder (up to
    1023 rows, PSUM-segmented at 512) to avoid a low-efficiency tiny block.
  - x / out DRAM layouts are block-contiguous so the per-block DMAs are
    single contiguous transfers (inputs on the sync HWDGE queue, output
    stores on the otherwise-idle gpsimd SWDGE queue).
"""

import ml_dtypes
import numpy as np

import concourse.bacc as bacc
import concourse.mybir as mybir
import concourse.tile as tile
from concourse import bass_utils

P = 128
D_IN, LAT, C = 512, 64, 8
H0, H1, H2 = 1024, 512, 256
N_CORES = 8
KX = D_IN // P
F32 = mybir.dt.float32
BF16 = mybir.dt.bfloat16
FP8 = mybir.dt.float8e4
AF = mybir.ActivationFunctionType
ALU = mybir.AluOpType
DR = mybir.MatmulPerfMode.DoubleRow
BF16_NP = ml_dtypes.bfloat16
FP8_NP = ml_dtypes.float8_e4m3

SX = 1.0 / 16.0   # fp8 scale on x (chosen so enc0's eviction scale is 1.0)
SW = 256.0        # fp8 scale on encoder weights
SA = 16.0         # fp8 scale on hidden activations
SEV0 = SA / (SX * SW)  # enc0 eviction scale == 1.0 -> 2-op DVE eligible
SEV = SA / (SA * SW)   # encu/enc2 eviction scale (2^-8)

# layer table: name -> (f_in, f_out, fp8)
LAYERS = dict(
    enc0=(D_IN, H0, True),
    encu=(H0, H1, True),
    enc2=(H1, H2, True),
    mu=(H2, LAT, True),
    lv=(H2, LAT, True),
    dec0=(LAT, H2, False),
    dec1=(H2, H1, False),
    dec2=(H1, H0, False),
    fin=(H0, D_IN, False),
)

DEC_TAIL = 4  # decoder groups emitted after the woven section


def _ceil_to(x, m):
    return ((x + m - 1) // m) * m


def _segs(nb, m):
    out = [m] * (nb // m)
    if nb % m:
        out.append(nb % m)
    return out


def _b2d(b, scale=1.0):
    """[f] bias -> [min(f,128), n_mtiles] (partition-major per m-tile)."""
    b = np.asarray(b, dtype=np.float32) * scale
    f = b.shape[0]
    if f >= P:
        return np.ascontiguousarray(b.reshape(f // P, P).T.astype(np.float32))
    return np.ascontiguousarray(b.reshape(1, f).T.astype(np.float32))


def _wpack(W, dt_np, scale=1.0):
    """[fi, fo] -> [min(fi,128), kt*fo] k-tile packed, cast to dt_np."""
    W = np.asarray(W, dtype=np.float32) * scale
    fi, fo = W.shape
    kp = min(P, fi)
    kt = max(1, fi // P)
    Wp = W.reshape(kt, kp, fo).transpose(1, 0, 2)
    Wp = np.clip(Wp, -240, 240).astype(dt_np)
    return np.ascontiguousarray(Wp.reshape(kp, kt * fo))


def _weave(*lists):
    """Merge lists of closures, interleaving by fractional progress."""
    lists = [l for l in lists if l]
    idx = [0] * len(lists)
    out = []
    while True:
        best, bp = -1, 2.0
        for j, l in enumerate(lists):
            if idx[j] < len(l):
                p = idx[j] / len(l)
                if p < bp:
                    best, bp = j, p
        if best < 0:
            return out
        out.append(lists[best][idx[best]])
        idx[best] += 1


def _build_module(npad, blocks):
    nc = bacc.Bacc("TRN2", target_bir_lowering=False, debug=False)

    dram = {}

    def din(name, shape, dt):
        dram[name] = nc.dram_tensor(name, list(shape), dt, kind="ExternalInput").ap()
        return dram[name]

    xT = din("xT", (P, KX * npad), FP8)
    epsT = din("epsT", (LAT, npad), F32)

    for name, (fi, fo, fp8) in LAYERS.items():
        kp = min(P, fi)
        kt = max(1, fi // P)
        din("w_" + name, (kp, kt * fo), FP8 if fp8 else BF16)
        din("b_" + name, (P if fo >= P else fo, max(1, fo // P)), F32)

    outT = nc.dram_tensor("outT", [P, KX * npad], BF16, kind="ExternalOutput").ap()

    with tile.TileContext(nc) as tc:
        with (
            tc.tile_pool(name="wpool", bufs=1) as wpool,
            tc.tile_pool(name="acts", bufs=2) as acts,
            tc.tile_pool(name="psum", bufs=8, space="PSUM") as psum,
        ):
            wsb = {}
            bsb = {}
            dma_rr = [0]

            def prologue_dma(out, in_):
                eng = nc.sync if dma_rr[0] % 2 == 0 else nc.scalar
                dma_rr[0] += 1
                eng.dma_start(out, in_)

            DEC_W = ("dec0", "dec1", "dec2", "fin")

            def load_weights(name):
                if name in wsb:
                    return
                fi, fo, fp8 = LAYERS[name]
                kp = min(P, fi)
                kt = max(1, fi // P)
                dt = FP8 if fp8 else BF16
                w_t = wpool.tile([kp, kt, fo], dt, tag=f"w_{name}", name=f"w_{name}")
                src = dram["w_" + name].rearrange("p (k f) -> p k f", k=kt)
                # spread encoder weight loads across the two input queues so
                # block 0's layers never wait behind a single queue
                weng = dict(enc0=nc.sync, encu=nc.scalar, enc2=nc.sync,
                            mu=nc.scalar, lv=nc.scalar)
                if name in DEC_W:
                    nc.gpsimd.dma_start(w_t[:], src)
                else:
                    weng[name].dma_start(w_t[:], src)
                bp = P if fo >= P else fo
                b_t = wpool.tile([bp, max(1, fo // P)], F32, tag=f"b_{name}", name=f"b_{name}")
                nc.gpsimd.dma_start(b_t[:], dram["b_" + name][:])
                wsb[name] = w_t
                bsb[name] = b_t

            def groups_fp8(lname, in_t, nb, func, scale, out_dt, evict="act"):
                """DoubleRow fp8 GEMM out = func(scale*(W.T @ in) + b).
                Returns (out_tile, [emit closures, one per m-tile])."""
                load_weights(lname)
                fi, fo, _ = LAYERS[lname]
                w_t, b_t = wsb[lname], bsb[lname]
                kt = fi // P
                n_m = max(1, fo // P)
                mp = min(P, fo)
                if fo >= P:
                    o_t = acts.tile([P, n_m, nb], out_dt, tag=f"h_{lname}", name=f"h_{lname}")
                else:
                    o_t = acts.tile([fo, nb], out_dt, tag=f"h_{lname}", name=f"h_{lname}")

                def mk(m):
                    def emit():
                        bias = b_t[:mp, m : m + 1]
                        s0 = 0
                        for seg in _segs(nb, 512):
                            ps = psum.tile([mp, seg], F32, tag="ps", name=f"ps_{lname}_{m}")
                            nkp = kt // 2
                            c0 = 0
                            for j, csz in enumerate(_segs(seg, 256)):
                                for i in range(nkp):
                                    nc.tensor.matmul(
                                        ps[:, c0 : c0 + csz],
                                        w_t[:, 2 * i : 2 * i + 2, m * mp : (m + 1) * mp],
                                        in_t[:, 2 * i : 2 * i + 2, s0 + c0 : s0 + c0 + csz],
                                        start=(i == 0),
                                        stop=(i == nkp - 1),
                                        perf_mode=DR,
                                    )
                                c0 += csz
                            dst = (o_t[:, m, s0 : s0 + seg] if fo >= P
                                   else o_t[:, s0 : s0 + seg])
                            if evict == "alt1" and m % 2 == 1:
                                # scale == 1.0: relu on DVE in 2 alu stages
                                nc.vector.tensor_scalar(dst, ps[:], bias, 0.0, ALU.add, ALU.max)
                            elif evict in ("act", "alt1"):
                                nc.scalar.activation(dst, ps[:], func, bias=bias, scale=scale)
                            else:  # dve identity: (ps * scale) + bias
                                nc.vector.tensor_scalar(dst, ps[:], scale, bias, ALU.mult, ALU.add)
                            s0 += seg
                    return emit

                return o_t, [mk(m) for m in range(n_m)]

            def groups_bf16(lname, in_t, nb, func, out_dt=BF16, out_tag=None):
                """bf16 GEMM; returns (out_tile, [closures])."""
                load_weights(lname)
                fi, fo, _ = LAYERS[lname]
                w_t, b_t = wsb[lname], bsb[lname]
                kt = max(1, fi // P)
                n_m = max(1, fo // P)
                mp = min(P, fo)
                tag = out_tag or f"h_{lname}"
                o_t = acts.tile([P, n_m, nb], out_dt, tag=tag, name=tag)

                def mk(m):
                    def emit():
                        bias = b_t[:mp, m : m + 1]
                        s0 = 0
                        for seg in _segs(nb, 512):
                            ps = psum.tile([mp, seg], F32, tag="ps", name=f"ps_{lname}_{m}")
                            for k in range(kt):
                                mov = (in_t[:, k, s0 : s0 + seg] if kt > 1
                                       else in_t[:, s0 : s0 + seg])
                                nc.tensor.matmul(
                                    ps[:],
                                    w_t[:, k, m * mp : (m + 1) * mp],
                                    mov,
                                    start=(k == 0),
                                    stop=(k == kt - 1),
                                )
                            dst = o_t[:, m, s0 : s0 + seg]
                            if m % 2 == 1:
                                if func is AF.Relu:
                                    nc.vector.tensor_scalar(dst, ps[:], bias, 0.0, ALU.add, ALU.max)
                                else:
                                    nc.vector.tensor_scalar(dst, ps[:], bias, None, ALU.add)
                            else:
                                nc.scalar.activation(
                                    dst, ps[:], func if func is not None else AF.Identity,
                                    bias=bias, scale=1.0,
                                )
                            s0 += seg
                    return emit

                return o_t, [mk(m) for m in range(n_m)]

            n_blk = len(blocks)
            offs = [sum(blocks[:i]) for i in range(n_blk)]
            x_in = [None] * n_blk
            eps_in = [None] * n_blk
            mu_sg = [None] * n_blk
            lat_out = [None] * n_blk

            def stage_load(b):
                # steady-state loads go on the scalar queue: the sync queue
                # carries output stores, which must not delay the x prefetch
                nb, off = blocks[b], offs[b]
                x_t = acts.tile([P, KX, nb], FP8, tag="x", bufs=3, name="x")
                src = xT[:, KX * off : KX * (off + nb)].rearrange("p (k n) -> p k n", k=KX)
                (prologue_dma if b == 0 else nc.scalar.dma_start)(x_t[:], src)
                e_t = acts.tile([LAT, nb], F32, tag="eps", bufs=3, name="e_t")
                (prologue_dma if b == 0 else nc.scalar.dma_start)(e_t[:], epsT[:, off : off + nb])
                x_in[b], eps_in[b] = x_t, e_t

            def build_enc(b):
                nb = blocks[b]
                h0, g0 = groups_fp8("enc0", x_in[b], nb, AF.Relu, SEV0, FP8, evict="alt1")
                h1, g1 = groups_fp8("encu", h0, nb, AF.Relu, SEV, FP8)
                h2, g2 = groups_fp8("enc2", h1, nb, AF.Relu, SEV, FP8)
                mu, gm = groups_fp8("mu", h2, nb, None, 1.0 / (SA * SW), F32, evict="dve")
                sg, gl = groups_fp8("lv", h2, nb, AF.Exp, 0.5 / (SA * SW), F32)
                mu_sg[b] = (mu, sg)
                return g0 + g1 + g2 + gm + gl

            def stage_lat(b):
                nb = blocks[b]
                mu, sg = mu_sg[b]
                tmp = acts.tile([LAT, nb], F32, tag="tmp", bufs=2, name="tmp")
                nc.vector.tensor_mul(tmp[:], sg[:], eps_in[b][:])
                z = acts.tile([LAT, nb], BF16, tag="z", bufs=2, name="z")
                nc.vector.tensor_add(z[:], tmp[:], mu[:])
                lat_out[b] = z

            def build_dec(b):
                nb, off = blocks[b], offs[b]
                h3, g3 = groups_bf16("dec0", lat_out[b], nb, AF.Relu)
                h4, g4 = groups_bf16("dec1", h3, nb, AF.Relu)
                h5, g5 = groups_bf16("dec2", h4, nb, AF.Relu)
                ot, g6 = groups_bf16("fin", h5, nb, None, out_dt=BF16, out_tag="out")

                # store each fin m-tile right after its eviction so the output
                # DMA pipelines with the remaining fin groups
                def mk_store(m):
                    def store():
                        eng = nc.sync if m % 2 == 0 else nc.gpsimd
                        eng.dma_start(
                            outT[:, KX * off + m * nb : KX * off + (m + 1) * nb],
                            ot[:, m, :],
                        )
                    return store

                g6s = []
                for m, g in enumerate(g6):
                    g6s += [g, mk_store(m)]
                return g3 + g4 + g5 + g6s

            # Warm up the PE (clock gate) with dummy matmuls while the
            # prologue DMAs stream in; rotate psum banks so they pipeline.
            wu_w = wpool.tile([P, P], BF16, tag="wu_w", name="wu_w")
            wu_x = wpool.tile([P, 512], BF16, tag="wu_x", name="wu_x")
            nc.vector.memset(wu_w[:], 0.0)
            nc.vector.memset(wu_x[:], 0.0)
            wu_ps = [psum.tile([P, 512], F32, tag="ps", name=f"wu_ps{i}") for i in range(3)]
            for i in range(15):
                nc.tensor.matmul(wu_ps[i % 3][:], wu_w[:], wu_x[:], start=True, stop=True)

            # software pipeline: weave encoder groups of block b with decoder
            # groups of block b-1; the decoder tail runs after the encoder so
            # mu/lv evictions + the z chain hide under decoder matmuls.
            # Load order on the input queues: x/eps(0) first, then encoder
            # weights, then the block-1 prefetch -- so block 0 never waits.
            stage_load(0)
            for name in ("enc0", "encu", "enc2", "mu", "lv"):
                load_weights(name)
            if n_blk > 1:
                stage_load(1)
            for g in build_enc(0):
                g()
            stage_lat(0)
            for b in range(1, n_blk):
                if b + 1 < n_blk:
                    stage_load(b + 1)
                dec = build_dec(b - 1)
                enc = build_enc(b)
                # block 1 only: give the encoder a head start, since block 0
                # has no decoder tail to hide its z-chain latency
                head = 8 if b == 1 else 0
                for g in enc[:head]:
                    g()
                for g in _weave(enc[head:], dec[:-DEC_TAIL]):
                    g()
                stage_lat(b)
                for g in dec[-DEC_TAIL:]:
                    g()
            for g in build_dec(n_blk - 1):
                g()

    nc.compile()
    return nc


def kernel(**inputs):
    x = np.asarray(inputs["x"], dtype=np.float32)
    lbl = np.asarray(inputs["cluster_labels"]).astype(np.int64)
    eps = np.asarray(inputs["eps"], dtype=np.float32)
    B = x.shape[0]

    counts = np.bincount(lbl, minlength=C)
    npad = max(512, _ceil_to(int(counts.max()), 64))
    n_full = max(0, npad // 512 - 1)
    blocks = [512] * n_full + [npad - 512 * n_full]

    rows = [np.nonzero(lbl == c)[0] for c in range(C)]
    offs = [sum(blocks[:i]) for i in range(len(blocks))]

    shared = {
        "w_enc0": _wpack(inputs["enc_W0"], FP8_NP, SW),
        "b_enc0": _b2d(inputs["enc_b0"], SA),
        "w_enc2": _wpack(inputs["enc_W2"], FP8_NP, SW),
        "b_enc2": _b2d(inputs["enc_b2"], SA),
        "w_mu": _wpack(inputs["mu_W"], FP8_NP, SW),
        "b_mu": _b2d(inputs["mu_b"]),
        "w_lv": _wpack(inputs["lv_W"], FP8_NP, SW),
        "b_lv": _b2d(inputs["lv_b"], 0.5),
        "w_dec1": _wpack(inputs["dec_W1"], BF16_NP),
        "b_dec1": _b2d(inputs["dec_b1"]),
    }

    in_maps = []
    for c in range(C):
        r = rows[c]
        xq = np.zeros((P, KX * npad), FP8_NP)
        xs = np.clip(
            x[r].T.reshape(KX, P, len(r)).transpose(1, 0, 2) * SX, -240, 240
        ).astype(FP8_NP)
        # block-contiguous layout: block b occupies cols [KX*off, KX*(off+nb))
        # with k-tile stride nb inside the block
        for off, nb in zip(offs, blocks):
            lo, hi = off, min(off + nb, len(r))
            if lo >= len(r):
                break
            seg = np.zeros((P, KX, nb), FP8_NP)
            seg[:, :, : hi - lo] = xs[:, :, lo:hi]
            xq[:, KX * off : KX * (off + nb)] = seg.reshape(P, -1)
        epsT = np.zeros((LAT, npad), np.float32)
        epsT[:, : len(r)] = eps[r].T
        m = dict(shared)
        m["xT"] = xq
        m["epsT"] = epsT
        for nm, W, b, bs in (
            ("encu", inputs["enc_Wu"][c], inputs["enc_bu"][c], SA),
            ("dec0", inputs["dec_Wu0"][c], inputs["dec_bu0"][c], 1.0),
            ("dec2", inputs["dec_Wu2"][c], inputs["dec_bu2"][c], 1.0),
            ("fin", inputs["fin_W"][c], inputs["fin_b"][c], 1.0),
        ):
            fp8 = LAYERS[nm][2]
            m["w_" + nm] = _wpack(W, FP8_NP if fp8 else BF16_NP, SW if fp8 else 1.0)
            m["b_" + nm] = _b2d(b, bs)
        in_maps.append(m)

    nc = _build_module(npad, blocks)
    res = bass_utils.run_bass_kernel_spmd(nc, in_maps, core_ids=list(range(N_CORES)))
    global LAST_RESULTS
    LAST_RESULTS = res

    out = np.empty((B, D_IN), np.float32)
    for c in range(C):
        r = rows[c]
        arr = np.asarray(res.results[c]["outT"]).astype(np.float32)
        for off, nb in zip(offs, blocks):
            lo, hi = off, min(off + nb, len(r))
            if lo >= len(r):
                break
            seg = arr[:, KX * off : KX * (off + nb)].reshape(P, KX, nb)[:, :, : hi - lo]
            out[r[lo:hi]] = seg.transpose(2, 1, 0).reshape(hi - lo, D_IN)
    return out
